# revision 24
# baseline (speedup 1.0000x reference)
"""Trainium2 Bass kernel for nn_DUDCLoss_1382979469646.

Data-parallel over the batch dim: 8 cores x 512 rows each. The loss is
factorized so each row needs only a handful of C-length passes, and the
eps=1e-5 inside log(q+eps) is dropped (rel err ~1.3e-3, tolerance 2e-2).

The device computes, per row, only the six C-length reductions
  E1 = sum exp(x1), E2 = sum exp(x2),
  G12 = sum exp(x1)*x2, G21 = sum exp(x2)*x1,
  M12 = sum sigmoid(x1)*logsigmoid(x2), M21 (accumulated as
        sum (r-1)*u = -M via r = 1/(1+exp(x)))
and exports them as a [128, 6T+2] tile. The host (which already holds
the gathered positive logits g) finishes the tiny [B,K] part in fp64.

Schedule: all exp/ln(1+A) passes stream first on ACT (paced by the two
DMA queues), the three s1=exp(u1) passes and the last tile's softplus
run at the end, so the Pool/DVE product chains drain in parallel with
ACT's late work and the final tail is one 512-col u-sub -> one 593ns
fused stt -> output DMA.

Engine balance per [128, 1024]-pair tile (ns):
  ACT : exp 1892, ln(1+A) 1892, s1=exp(u1) 1038 (tiles 0..2)
  Pool: u = x - ln(1+A) 1707, bf16 product mults 853 each
  DVE : E reduces via 4x-mode tensor_scalar+accum (327 each, B = 1+A
        folded in via op0=add), r=recip(B) 1127, G/M12 reduces 327,
        M21 fused stt (r2-1)*u1 1127
"""

import numpy as np

NCORES = 8
B, C, K = 4096, 1024, 8
RPC = B // NCORES          # rows per core
P = 128                    # partitions
T = RPC // P               # row-tiles per core
TK = T * K
EPS = 1e-5
NOUT = 6 * T + 2           # E1,E2,G12,G21,M12,M21neg (T each) + tile3 halves

_cache = {}


def _patch_act_tables(mybir, bacc):
    """Make the ACT-table-load inserter resolve both Exp and Ln to the one
    set that holds both (natural_log_exp_and_others). The default policy
    picks a singleton set per function, inserting a ~1.3us table load at
    every Exp<->Ln transition in the scheduled stream."""
    if getattr(bacc, "_dudc_act_patch", False):
        return
    orig = bacc.get_activation_tables
    both = {mybir.ActivationFunctionType.Exp, mybir.ActivationFunctionType.Ln}

    def patched(arch):
        tabs = orig(arch)
        if any(both <= funcs for funcs in tabs.values()):
            for name, funcs in tabs.items():
                if not both <= funcs:
                    funcs.difference_update(both)
        return tabs

    bacc.get_activation_tables = patched
    bacc._dudc_act_patch = True


def _build():
    import concourse.bass as bass
    import concourse.tile as tile
    from concourse import bacc, mybir

    _patch_act_tables(mybir, bacc)

    fp32 = mybir.dt.float32
    bf16 = mybir.dt.bfloat16
    AF = mybir.ActivationFunctionType
    ALU = mybir.AluOpType

    nc = bacc.Bacc(
        "TRN2",
        target_bir_lowering=False,
        debug=False,
        num_devices=NCORES,
    )

    x1d = nc.dram_tensor("x1", [RPC, C], fp32, kind="ExternalInput").ap()
    x2d = nc.dram_tensor("x2", [RPC, C], fp32, kind="ExternalInput").ap()
    outd = nc.dram_tensor("out", [P, NOUT], fp32, kind="ExternalOutput").ap()

    H = C // 2
    LAST = T - 1

    with tile.TileContext(nc) as tc:
        with (
            tc.tile_pool(name="x", bufs=T) as xp,
            tc.tile_pool(name="A", bufs=2) as ap_,
            tc.tile_pool(name="llp", bufs=2) as llpp,
            tc.tile_pool(name="u", bufs=T) as up,
            tc.tile_pool(name="br", bufs=2) as brp,
            tc.tile_pool(name="sg", bufs=2) as sgp,
            tc.tile_pool(name="pr", bufs=3) as prp,
            tc.tile_pool(name="small", bufs=1) as sm,
        ):
            # out columns: [E1+C | E2+C | G12 | G21 | M12 | M21neg] x T,
            # then [M12neg_h1, M21neg_h1] for the last tile's second half
            outt = sm.tile([P, NOUT], fp32)

            # primer: hoist the ~1.3us ACT table load to t=0
            dm = sm.tile([P, 1], fp32)
            dmo = sm.tile([P, 1], fp32)
            nc.vector.memset(dm[:], 0.0)
            nc.scalar.activation(dmo[:], dm[:], AF.Exp)

            def red(acc_slot, src, n=C):
                # free-axis sum at 4x rate: ts (x*1), reduce-add seeded 0
                scr = prp.tile([P, C], bf16, tag="red")
                nc.vector.tensor_scalar(
                    scr[:, 0:n], src, 1.0, 0.0, op0=ALU.mult, op1=ALU.add,
                    accum_out=acc_slot,
                )

            def red_fold(acc_slot, src, bout):
                # bout = src + 1 (=B); reduce-add seeded 0: accum = E + C
                nc.vector.tensor_scalar(
                    bout, src, 1.0, 0.0, op0=ALU.add, op1=ALU.add,
                    accum_out=acc_slot,
                )

            # ---------------- DMA + exp/softplus streams ----------------
            xts = []       # (x1 slice, x2 slice)
            Ats = []
            LLps = []
            uts = []
            Rts = []       # r2 per tile; tile LAST also gets r1

            for t in range(T):
                r0, r1 = t * P, (t + 1) * P
                if t == 0:
                    xta = xp.tile([P, C], fp32, tag="xa")
                    xtb = xp.tile([P, C], fp32, tag="xb")
                    nc.sync.dma_start(xta[:, 0:H], x1d[r0:r1, 0:H])
                    nc.sync.dma_start(xta[:, H:C], x1d[r0:r1, H:C])
                    nc.gpsimd.dma_start(xtb[:], x2d[r0:r1, :])
                    xts.append((xta[:], xtb[:], None))
                else:
                    xt = xp.tile([P, 2 * C], fp32, tag="x")
                    nc.sync.dma_start(xt[:, 0:C], x1d[r0:r1, :])
                    nc.sync.dma_start(xt[:, C : 2 * C], x2d[r0:r1, :])
                    xts.append((xt[:, 0:C], xt[:, C : 2 * C], xt))

            for t in range(T):
                x1s, x2s, xfull = xts[t]
                if t < 2:
                    At = ap_.tile([P, 2 * C], bf16, tag="A")
                else:
                    At = ap_.tile([P, 2 * C], bf16, tag="A2")
                # ---- ACT: exp ----
                if t == 0:
                    nc.scalar.activation(At[:, 0:H], x1s[:, 0:H], AF.Exp)
                    nc.scalar.activation(At[:, H:C], x1s[:, H:C], AF.Exp)
                    nc.scalar.activation(At[:, C : 2 * C], x2s, AF.Exp)
                else:
                    nc.scalar.activation(At[:], xfull[:], AF.Exp)
                Ats.append(At)

                # ---- Pool: G product mults ----
                pg1 = prp.tile([P, C], bf16, tag="pg1")
                nc.gpsimd.tensor_tensor(pg1[:], At[:, 0:C], x2s, op=ALU.mult)
                pg2 = prp.tile([P, C], bf16, tag="pg2")
                nc.gpsimd.tensor_tensor(
                    pg2[:], At[:, C : 2 * C], x1s, op=ALU.mult
                )

                # ---- DVE: E sums (B folded), recips, G reduces ----
                B1t = brp.tile([P, C], bf16, tag="B1")
                red_fold(outt[:, t : t + 1], At[:, 0:C], B1t[:])
                B2t = brp.tile([P, C], bf16, tag="B2")
                red_fold(outt[:, T + t : T + t + 1], At[:, C : 2 * C], B2t[:])
                R2t = brp.tile([P, C], bf16, tag="R2")
                with nc.allow_low_precision("r feeds bf16 products"):
                    nc.vector.reciprocal(R2t[:], B2t[:])
                    if t == LAST:
                        R1t = brp.tile([P, C], bf16, tag="R1")
                        nc.vector.reciprocal(R1t[:], B1t[:])
                Rts.append(R2t)
                red(outt[:, 2 * T + t : 2 * T + t + 1], pg1[:])
                red(outt[:, 3 * T + t : 3 * T + t + 1], pg2[:])

                # ---- ACT: softplus; Pool: u; DVE: fused M21 ----
                ut = up.tile([P, 2 * C], bf16, tag="u")
                if t < LAST:
                    LLpt = llpp.tile([P, 2 * C], fp32, tag="llp")
                    nc.scalar.activation(LLpt[:], At[:], AF.Ln, bias=1.0)
                    nc.gpsimd.tensor_sub(ut[:, 0:C], x1s, LLpt[:, 0:C])
                    nc.gpsimd.tensor_sub(
                        ut[:, C : 2 * C], x2s, LLpt[:, C : 2 * C]
                    )
                    scm = prp.tile([P, C], bf16, tag="scm")
                    nc.vector.scalar_tensor_tensor(
                        scm[:], R2t[:], 1.0, ut[:, 0:C],
                        op0=ALU.subtract, op1=ALU.mult,
                        accum_out=outt[:, 5 * T + t : 5 * T + t + 1],
                    )
                uts.append(ut)

            # ---------------- late chains ----------------
            # s1 = exp(u1) for tiles 0..T-2; products on Pool; reduces DVE
            sgts = []
            for t in range(LAST):
                sgt = sgp.tile([P, C], bf16, tag="sg")
                nc.scalar.activation(sgt[:], uts[t][:, 0:C], AF.Exp)
                sgts.append(sgt)

            # last tile softplus: x1 full first (gates M21), then x2 halves
            x1s, x2s, _ = xts[LAST]
            At = Ats[LAST]
            ut = uts[LAST]
            LLpt = llpp.tile([P, 2 * C], fp32, tag="llpL")
            nc.scalar.activation(LLpt[:, 0:C], At[:, 0:C], AF.Ln, bias=1.0)
            nc.gpsimd.tensor_sub(ut[:, 0:H], x1s[:, 0:H], LLpt[:, 0:H])
            nc.gpsimd.tensor_sub(ut[:, H:C], x1s[:, H:C], LLpt[:, H:C])
            nc.scalar.activation(
                LLpt[:, C : C + H], At[:, C : C + H], AF.Ln, bias=1.0
            )
            nc.scalar.activation(
                LLpt[:, C + H : 2 * C], At[:, C + H : 2 * C], AF.Ln, bias=1.0
            )

            # Pool: pm products for the sg tiles
            pms = []
            for t in range(LAST):
                pm = prp.tile([P, C], bf16, tag="pm")
                nc.gpsimd.tensor_tensor(
                    pm[:], sgts[t][:], uts[t][:, C : 2 * C], op=ALU.mult
                )
                pms.append(pm)
            # Pool: last tile u2 halves
            nc.gpsimd.tensor_sub(
                ut[:, C : C + H], x2s[:, 0:H], LLpt[:, C : C + H]
            )
            nc.gpsimd.tensor_sub(
                ut[:, C + H : 2 * C], x2s[:, H:C], LLpt[:, C + H : 2 * C]
            )

            # DVE: M12 reduces for sg tiles, then the last tile's fused Ms
            for t in range(LAST):
                red(outt[:, 4 * T + t : 4 * T + t + 1], pms[t][:])
            R2L = Rts[LAST]
            scmL = prp.tile([P, 2 * C], bf16, tag="scmL")
            nc.vector.scalar_tensor_tensor(
                scmL[:, 0:H], R2L[:, 0:H], 1.0, ut[:, 0:H],
                op0=ALU.subtract, op1=ALU.mult,
                accum_out=outt[:, 5 * T + LAST : 5 * T + LAST + 1],
            )
            nc.vector.scalar_tensor_tensor(
                scmL[:, H:C], R2L[:, H:C], 1.0, ut[:, H:C],
                op0=ALU.subtract, op1=ALU.mult,
                accum_out=outt[:, 6 * T + 1 : 6 * T + 2],
            )
            nc.vector.scalar_tensor_tensor(
                scmL[:, C : C + H], R1t[:, 0:H], 1.0, ut[:, C : C + H],
                op0=ALU.subtract, op1=ALU.mult,
                accum_out=outt[:, 4 * T + LAST : 4 * T + LAST + 1],
            )
            nc.vector.scalar_tensor_tensor(
                scmL[:, C + H : 2 * C], R1t[:, H:C], 1.0, ut[:, C + H : 2 * C],
                op0=ALU.subtract, op1=ALU.mult,
                accum_out=outt[:, 6 * T : 6 * T + 1],
            )

            nc.sync.dma_start(outd, outt[:])

    nc.compile()
    return nc


def _get_nc():
    if "nc" not in _cache:
        _cache["nc"] = _build()
    return _cache["nc"]


def kernel(out1, out2, para, target, pos_idx):
    from concourse.bass_utils import run_bass_kernel_spmd

    nc = _get_nc()

    out1 = np.ascontiguousarray(out1, dtype=np.float32)
    out2 = np.ascontiguousarray(out2, dtype=np.float32)
    idx = pos_idx.astype(np.int64)
    g1 = np.take_along_axis(out1, idx, axis=1).astype(np.float64)  # [B, K]
    g2 = np.take_along_axis(out2, idx, axis=1).astype(np.float64)

    in_maps = [
        {
            "x1": out1[c * RPC : (c + 1) * RPC],
            "x2": out2[c * RPC : (c + 1) * RPC],
        }
        for c in range(NCORES)
    ]
    res = run_bass_kernel_spmd(nc, in_maps, core_ids=list(range(NCORES)))
    parts = np.stack([r["out"] for r in res.results])  # [NCORES, P, NOUT]

    # unpack: col q*T+t of row p is global row c*RPC + t*P + p
    main = parts[:, :, : 6 * T].reshape(NCORES, P, 6, T)
    q = main.transpose(0, 3, 1, 2).reshape(B, 6).astype(np.float64)
    E1, E2, G12, G21, M12, M21n = (q[:, i] for i in range(6))
    E1 = E1 - C          # B-fold adds C to the E accumulators
    E2 = E2 - C
    # last tile: M12 accumulated as -M12 (fold), second halves in the two
    # extra columns
    extra = parts[:, :, 6 * T : 6 * T + 2].astype(np.float64)  # [NC, P, 2]
    for c in range(NCORES):
        sl = slice(c * RPC + (T - 1) * P, c * RPC + T * P)
        M12[sl] = -(M12[sl] + extra[c, :, 0])
        M21n[sl] = M21n[sl] + extra[c, :, 1]
    M21 = -M21n

    # host finale in fp64 (tiny [B,K] math)
    a1 = np.exp(g1)
    a2 = np.exp(g2)
    D1 = (E1 - a1.sum(1))[:, None] + a1
    D2 = (E2 - a2.sum(1))[:, None] + a2
    P12 = (a1 * g2).sum(1)
    P21 = (a2 * g1).sum(1)
    row_single = (
        np.log(D1).sum(1) + np.log(D2).sum(1)
        - (G12 - P12) * (1.0 / D1).sum(1) - (a1 * g2 / D1).sum(1)
        - (G21 - P21) * (1.0 / D2).sum(1) - (a2 * g1 / D2).sum(1)
    )
    single = row_single.sum() / (B * K)
    multi = -(M12.sum() + M21.sum()) / B
    p = float(np.asarray(para))
    return np.asarray(p * multi + (1.0 - p) * single, dtype=np.float32)


# revision 26
# speedup vs baseline: 1.0324x; 1.0324x over previous
"""Trainium2 Bass kernel for nn_DUDCLoss_1382979469646.

Data-parallel over the batch dim: 8 cores x 512 rows each. The loss is
factorized so each row needs only a handful of C-length passes, and the
eps=1e-5 inside log(q+eps) is dropped (rel err ~1.3e-3, tolerance 2e-2).

The device computes, per row, only the six C-length reductions
  E1 = sum exp(x1), E2 = sum exp(x2),
  G12 = sum exp(x1)*x2, G21 = sum exp(x2)*x1,
  M12 = sum sigmoid(x1)*logsigmoid(x2), M21 (accumulated as
        sum (r-1)*u = -M via r = 1/(1+exp(x)))
and exports them as a [128, 6T+2] tile. The host (which already holds
the gathered positive logits g) finishes the tiny [B,K] part in fp64.

Schedule: all exp/ln(1+A) passes stream first on ACT (paced by the two
DMA queues), the three s1=exp(u1) passes and the last tile's softplus
run at the end, so the Pool/DVE product chains drain in parallel with
ACT's late work and the final tail is one 512-col u-sub -> one 593ns
fused stt -> output DMA.

Engine balance per [128, 1024]-pair tile (ns):
  ACT : exp 1892, ln(1+A) 1892, s1=exp(u1) 1038 (tiles 0..2)
  Pool: u = x - ln(1+A) 1707, bf16 product mults 853 each
  DVE : E reduces via 4x-mode tensor_scalar+accum (327 each, B = 1+A
        folded in via op0=add), r=recip(B) 1127, G/M12 reduces 327,
        M21 fused stt (r2-1)*u1 1127
"""

import numpy as np

NCORES = 8
B, C, K = 4096, 1024, 8
RPC = B // NCORES          # rows per core
P = 128                    # partitions
T = RPC // P               # row-tiles per core
TK = T * K
EPS = 1e-5
NOUT = 6 * T + 2           # E1,E2,G12,G21,M12,M21neg (T each) + tile3 halves

_cache = {}


def _patch_act_tables(mybir, bacc):
    """Make the ACT-table-load inserter resolve both Exp and Ln to the one
    set that holds both (natural_log_exp_and_others). The default policy
    picks a singleton set per function, inserting a ~1.3us table load at
    every Exp<->Ln transition in the scheduled stream."""
    if getattr(bacc, "_dudc_act_patch", False):
        return
    orig = bacc.get_activation_tables
    both = {mybir.ActivationFunctionType.Exp, mybir.ActivationFunctionType.Ln}

    def patched(arch):
        tabs = orig(arch)
        if any(both <= funcs for funcs in tabs.values()):
            for name, funcs in tabs.items():
                if not both <= funcs:
                    funcs.difference_update(both)
        return tabs

    bacc.get_activation_tables = patched
    bacc._dudc_act_patch = True


def _build():
    import concourse.bass as bass
    import concourse.tile as tile
    from concourse import bacc, mybir

    _patch_act_tables(mybir, bacc)

    fp32 = mybir.dt.float32
    bf16 = mybir.dt.bfloat16
    AF = mybir.ActivationFunctionType
    ALU = mybir.AluOpType

    nc = bacc.Bacc(
        "TRN2",
        target_bir_lowering=False,
        debug=False,
        num_devices=NCORES,
    )

    x1d = nc.dram_tensor("x1", [RPC, C], fp32, kind="ExternalInput").ap()
    x2d = nc.dram_tensor("x2", [RPC, C], fp32, kind="ExternalInput").ap()
    outd = nc.dram_tensor("out", [P, NOUT], fp32, kind="ExternalOutput").ap()

    H = C // 2
    LAST = T - 1

    with tile.TileContext(nc) as tc:
        with (
            tc.tile_pool(name="x", bufs=T) as xp,
            tc.tile_pool(name="A", bufs=2) as ap_,
            tc.tile_pool(name="llp", bufs=2) as llpp,
            tc.tile_pool(name="u", bufs=T) as up,
            tc.tile_pool(name="br", bufs=2) as brp,
            tc.tile_pool(name="sg", bufs=2) as sgp,
            tc.tile_pool(name="pr", bufs=3) as prp,
            tc.tile_pool(name="small", bufs=1) as sm,
        ):
            # out columns: [E1+C | E2+C | G12 | G21 | M12 | M21neg] x T,
            # then [M12neg_h1, M21neg_h1] for the last tile's second half
            outt = sm.tile([P, NOUT], fp32)

            # primer: hoist the ~1.3us ACT table load to t=0
            dm = sm.tile([P, 1], fp32)
            dmo = sm.tile([P, 1], fp32)
            nc.vector.memset(dm[:], 0.0)
            nc.scalar.activation(dmo[:], dm[:], AF.Exp)

            def red(acc_slot, src, n=C):
                # free-axis sum at 4x rate: ts (x*1), reduce-add seeded 0
                scr = prp.tile([P, C], bf16, tag="red")
                nc.vector.tensor_scalar(
                    scr[:, 0:n], src, 1.0, 0.0, op0=ALU.mult, op1=ALU.add,
                    accum_out=acc_slot,
                )

            def red_fold(acc_slot, src, bout):
                # bout = src + 1 (=B); reduce-add seeded 0: accum = E + C
                nc.vector.tensor_scalar(
                    bout, src, 1.0, 0.0, op0=ALU.add, op1=ALU.add,
                    accum_out=acc_slot,
                )

            # ---------------- DMA + exp/softplus streams ----------------
            xts = []       # (x1 slice, x2 slice)
            Ats = []
            LLps = []
            uts = []
            Rts = []       # r2 per tile; tile LAST also gets r1

            for t in range(T):
                r0, r1 = t * P, (t + 1) * P
                if t == 0:
                    xta = xp.tile([P, C], fp32, tag="xa")
                    xtb = xp.tile([P, C], fp32, tag="xb")
                    nc.sync.dma_start(xta[:, 0:H], x1d[r0:r1, 0:H])
                    nc.sync.dma_start(xta[:, H:C], x1d[r0:r1, H:C])
                    nc.gpsimd.dma_start(xtb[:], x2d[r0:r1, :])
                    xts.append((xta[:], xtb[:], None))
                else:
                    xt = xp.tile([P, 2 * C], fp32, tag="x")
                    nc.sync.dma_start(xt[:, 0:C], x1d[r0:r1, :])
                    q2 = nc.gpsimd if t == 2 else nc.sync
                    q2.dma_start(xt[:, C : 2 * C], x2d[r0:r1, :])
                    xts.append((xt[:, 0:C], xt[:, C : 2 * C], xt))

            for t in range(T):
                x1s, x2s, xfull = xts[t]
                if t < 2:
                    At = ap_.tile([P, 2 * C], bf16, tag="A")
                else:
                    At = ap_.tile([P, 2 * C], bf16, tag="A2")
                # ---- ACT: exp ----
                if t == 0:
                    nc.scalar.activation(At[:, 0:H], x1s[:, 0:H], AF.Exp)
                    nc.scalar.activation(At[:, H:C], x1s[:, H:C], AF.Exp)
                    nc.scalar.activation(At[:, C : 2 * C], x2s, AF.Exp)
                else:
                    nc.scalar.activation(At[:], xfull[:], AF.Exp)
                Ats.append(At)

                # ---- Pool: G product mults ----
                pg1 = prp.tile([P, C], bf16, tag="pg1")
                nc.gpsimd.tensor_tensor(pg1[:], At[:, 0:C], x2s, op=ALU.mult)
                pg2 = prp.tile([P, C], bf16, tag="pg2")
                nc.gpsimd.tensor_tensor(
                    pg2[:], At[:, C : 2 * C], x1s, op=ALU.mult
                )

                # ---- DVE: E sums (B folded), recips, G reduces ----
                B1t = brp.tile([P, C], bf16, tag="B1")
                red_fold(outt[:, t : t + 1], At[:, 0:C], B1t[:])
                B2t = brp.tile([P, C], bf16, tag="B2")
                red_fold(outt[:, T + t : T + t + 1], At[:, C : 2 * C], B2t[:])
                R2t = brp.tile([P, C], bf16, tag="R2")
                with nc.allow_low_precision("r feeds bf16 products"):
                    nc.vector.reciprocal(R2t[:], B2t[:])
                    if t == LAST:
                        R1t = brp.tile([P, C], bf16, tag="R1")
                        nc.vector.reciprocal(R1t[:], B1t[:])
                Rts.append(R2t)
                red(outt[:, 2 * T + t : 2 * T + t + 1], pg1[:])
                red(outt[:, 3 * T + t : 3 * T + t + 1], pg2[:])

                # ---- ACT: softplus; Pool: u; DVE: fused M21 ----
                ut = up.tile([P, 2 * C], bf16, tag="u")
                if t < LAST:
                    LLpt = llpp.tile([P, 2 * C], fp32, tag="llp")
                    nc.scalar.activation(LLpt[:], At[:], AF.Ln, bias=1.0)
                    nc.gpsimd.tensor_sub(ut[:, 0:C], x1s, LLpt[:, 0:C])
                    nc.gpsimd.tensor_sub(
                        ut[:, C : 2 * C], x2s, LLpt[:, C : 2 * C]
                    )
                    scm = prp.tile([P, C], bf16, tag="scm")
                    nc.vector.scalar_tensor_tensor(
                        scm[:], R2t[:], 1.0, ut[:, 0:C],
                        op0=ALU.subtract, op1=ALU.mult,
                        accum_out=outt[:, 5 * T + t : 5 * T + t + 1],
                    )
                uts.append(ut)

            # ---------------- late chains ----------------
            # s1 = exp(u1) for tiles 0..T-2; products on Pool; reduces DVE
            sgts = []
            for t in range(LAST):
                sgt = sgp.tile([P, C], bf16, tag="sg")
                nc.scalar.activation(sgt[:], uts[t][:, 0:C], AF.Exp)
                sgts.append(sgt)

            # last tile softplus: x1 full first (gates M21), then x2 halves
            x1s, x2s, _ = xts[LAST]
            At = Ats[LAST]
            ut = uts[LAST]
            LLpt = llpp.tile([P, 2 * C], fp32, tag="llpL")
            nc.scalar.activation(LLpt[:, 0:C], At[:, 0:C], AF.Ln, bias=1.0)
            nc.gpsimd.tensor_sub(ut[:, 0:H], x1s[:, 0:H], LLpt[:, 0:H])
            nc.gpsimd.tensor_sub(ut[:, H:C], x1s[:, H:C], LLpt[:, H:C])
            nc.scalar.activation(
                LLpt[:, C : C + H], At[:, C : C + H], AF.Ln, bias=1.0
            )
            nc.scalar.activation(
                LLpt[:, C + H : 2 * C], At[:, C + H : 2 * C], AF.Ln, bias=1.0
            )

            # Pool: pm products for the first sg tiles, then the last tile's
            # u2 halves (ahead of the final sg tile's pm so the tail chain
            # u2 -> fused M12 isn't queued behind it)
            pms = []
            for t in range(LAST - 1):
                pm = prp.tile([P, C], bf16, tag="pm")
                nc.gpsimd.tensor_tensor(
                    pm[:], sgts[t][:], uts[t][:, C : 2 * C], op=ALU.mult
                )
                pms.append(pm)
            nc.gpsimd.tensor_sub(
                ut[:, C : C + H], x2s[:, 0:H], LLpt[:, C : C + H]
            )
            nc.gpsimd.tensor_sub(
                ut[:, C + H : 2 * C], x2s[:, H:C], LLpt[:, C + H : 2 * C]
            )
            pmL = prp.tile([P, C], bf16, tag="pm")
            nc.gpsimd.tensor_tensor(
                pmL[:], sgts[LAST - 1][:], uts[LAST - 1][:, C : 2 * C],
                op=ALU.mult,
            )
            pms.append(pmL)

            # DVE: M12 reduces for sg tiles, then the last tile's fused Ms
            for t in range(LAST):
                red(outt[:, 4 * T + t : 4 * T + t + 1], pms[t][:])
            R2L = Rts[LAST]
            scmL = prp.tile([P, 2 * C], bf16, tag="scmL")
            nc.vector.scalar_tensor_tensor(
                scmL[:, 0:H], R2L[:, 0:H], 1.0, ut[:, 0:H],
                op0=ALU.subtract, op1=ALU.mult,
                accum_out=outt[:, 5 * T + LAST : 5 * T + LAST + 1],
            )
            nc.vector.scalar_tensor_tensor(
                scmL[:, H:C], R2L[:, H:C], 1.0, ut[:, H:C],
                op0=ALU.subtract, op1=ALU.mult,
                accum_out=outt[:, 6 * T + 1 : 6 * T + 2],
            )
            nc.vector.scalar_tensor_tensor(
                scmL[:, C : C + H], R1t[:, 0:H], 1.0, ut[:, C : C + H],
                op0=ALU.subtract, op1=ALU.mult,
                accum_out=outt[:, 4 * T + LAST : 4 * T + LAST + 1],
            )
            nc.vector.scalar_tensor_tensor(
                scmL[:, C + H : 2 * C], R1t[:, H:C], 1.0, ut[:, C + H : 2 * C],
                op0=ALU.subtract, op1=ALU.mult,
                accum_out=outt[:, 6 * T : 6 * T + 1],
            )

            nc.sync.dma_start(outd, outt[:])

    nc.compile()
    return nc


def _get_nc():
    if "nc" not in _cache:
        _cache["nc"] = _build()
    return _cache["nc"]


def kernel(out1, out2, para, target, pos_idx):
    from concourse.bass_utils import run_bass_kernel_spmd

    nc = _get_nc()

    out1 = np.ascontiguousarray(out1, dtype=np.float32)
    out2 = np.ascontiguousarray(out2, dtype=np.float32)
    idx = pos_idx.astype(np.int64)
    g1 = np.take_along_axis(out1, idx, axis=1).astype(np.float64)  # [B, K]
    g2 = np.take_along_axis(out2, idx, axis=1).astype(np.float64)

    in_maps = [
        {
            "x1": out1[c * RPC : (c + 1) * RPC],
            "x2": out2[c * RPC : (c + 1) * RPC],
        }
        for c in range(NCORES)
    ]
    res = run_bass_kernel_spmd(nc, in_maps, core_ids=list(range(NCORES)))
    parts = np.stack([r["out"] for r in res.results])  # [NCORES, P, NOUT]

    # unpack: col q*T+t of row p is global row c*RPC + t*P + p
    main = parts[:, :, : 6 * T].reshape(NCORES, P, 6, T)
    q = main.transpose(0, 3, 1, 2).reshape(B, 6).astype(np.float64)
    E1, E2, G12, G21, M12, M21n = (q[:, i] for i in range(6))
    E1 = E1 - C          # B-fold adds C to the E accumulators
    E2 = E2 - C
    # last tile: M12 accumulated as -M12 (fold), second halves in the two
    # extra columns
    extra = parts[:, :, 6 * T : 6 * T + 2].astype(np.float64)  # [NC, P, 2]
    for c in range(NCORES):
        sl = slice(c * RPC + (T - 1) * P, c * RPC + T * P)
        M12[sl] = -(M12[sl] + extra[c, :, 0])
        M21n[sl] = M21n[sl] + extra[c, :, 1]
    M21 = -M21n

    # host finale in fp64 (tiny [B,K] math)
    a1 = np.exp(g1)
    a2 = np.exp(g2)
    D1 = (E1 - a1.sum(1))[:, None] + a1
    D2 = (E2 - a2.sum(1))[:, None] + a2
    P12 = (a1 * g2).sum(1)
    P21 = (a2 * g1).sum(1)
    row_single = (
        np.log(D1).sum(1) + np.log(D2).sum(1)
        - (G12 - P12) * (1.0 / D1).sum(1) - (a1 * g2 / D1).sum(1)
        - (G21 - P21) * (1.0 / D2).sum(1) - (a2 * g1 / D2).sum(1)
    )
    single = row_single.sum() / (B * K)
    multi = -(M12.sum() + M21.sum()) / B
    p = float(np.asarray(para))
    return np.asarray(p * multi + (1.0 - p) * single, dtype=np.float32)
